# revision 1
# baseline (speedup 1.0000x reference)
from contextlib import ExitStack

import numpy as np

import concourse.bass as bass
from concourse import bacc
import concourse.mybir as mybir
import concourse.tile as tile
from concourse.bass_utils import run_bass_kernel_spmd

H = 128
D_IN = 256
N_CORES = 8
F32 = mybir.dt.float32
F32R = mybir.dt.float32r
AF = mybir.ActivationFunctionType


def _r(ap):
    return ap.bitcast(F32R)


def build_gru(nc, B, T, L, GRP, has_bias, has_bhh):
    NG = T // GRP
    assert T % GRP == 0

    CW = L * 2 * 2 * 3 * H
    CU = L * 2 * 3 * H
    c_u = CW
    c_wd = CW + CU
    c_bias = c_wd + 2
    c_bhh = c_bias + CU
    c_ones = c_bhh + L * H
    c_ind2 = c_ones + GRP * B
    c_h0 = c_ind2 + 2 * B
    C = c_h0 + 2 * B
    x = nc.dram_tensor("x", [D_IN, T * B], F32R, kind="ExternalInput")
    wpack = nc.dram_tensor("wpack", [H, C], F32R, kind="ExternalInput")
    y = nc.dram_tensor("y", [1, B], F32, kind="ExternalOutput")

    with tile.TileContext(nc) as tc, ExitStack() as ctx:
        const = ctx.enter_context(tc.tile_pool(name="const", bufs=1))
        rhsp = ctx.enter_context(tc.tile_pool(name="rhsp", bufs=3))
        outp = ctx.enter_context(tc.tile_pool(name="outp", bufs=3))
        stepp = ctx.enter_context(tc.tile_pool(name="stepp", bufs=6))
        psum = ctx.enter_context(tc.tile_pool(name="psum", bufs=1,
                                              space="PSUM"))
        pscr = ctx.enter_context(tc.tile_pool(name="pscr", bufs=2,
                                              space="PSUM"))
        dramp = ctx.enter_context(tc.tile_pool(name="dramp", bufs=1,
                                               space="DRAM"))

        seqs = []
        for p in "AB":
            sf = dramp.tile([H, T * B], F32R, name=f"seq{p}f", tag=f"seq{p}f")
            sb = dramp.tile([H, T * B], F32R, name=f"seq{p}b", tag=f"seq{p}b")
            seqs.append((sf, sb))

        pk = const.tile([H, C], F32R)
        nc.sync.dma_start(out=pk, in_=wpack[:])

        def w_ap(l, d, k, gi):
            c = ((l * 2 + d) * 2 + k) * 3 * H + gi * H
            return pk[:, c:c + H]

        def u_ap(l, d, gi):
            c = c_u + (l * 2 + d) * 3 * H + gi * H
            return pk[:, c:c + H]

        def wd_ap(d):
            return pk[:, c_wd + d:c_wd + d + 1]

        def bias_ap(l, d, gi):
            c = c_bias + (l * 2 + d) * 3 * H + gi * H
            return pk[0:1, c:c + H]

        def bhh_ap(l):
            return pk[0:2, c_bhh + l * H:c_bhh + (l + 1) * H]

        h0_sb = pk[:, c_h0:c_h0 + 2 * B].rearrange("p (d b) -> p d b", d=2)
        ones_sb = pk[0:1, c_ones:c_ones + GRP * B]
        ind2_sb = pk[0:2, c_ind2:c_ind2 + 2 * B]

        prev_out = None
        outbuf = None

        def pair2(tile4, cf, cb):
            ps = tile4.ap[0][0]
            return bass.AP(tensor=tile4.tensor,
                           offset=tile4.offset + cf * B,
                           ap=[[ps, H], [(GRP + cb - cf) * B, 2], [1, B]])

        for l in range(L):
            for g in range(NG):
                rhs = {}
                for d, dn in ((0, "f"), (1, "b")):
                    t_lo = GRP * g if d == 0 else T - GRP * (g + 1)
                    for k in range(2):
                        rt = rhsp.tile([H, GRP, B], F32R, tag=f"rhs{dn}{k}",
                                       name=f"rhs_{dn}{k}_{l}_{g}")
                        if l == 0:
                            s_fb = x[:][k * H:(k + 1) * H, :]
                        else:
                            s_fb = seqs[(l - 1) % 2][k]
                        src = s_fb.rearrange("p (t b) -> p t b", b=B)[
                            :, t_lo:t_lo + GRP, :]
                        nc.sync.dma_start(out=rt, in_=src)
                        rhs[(d, k)] = rt

                zrb = psum.tile([H, 4, GRP, B], F32, tag="zrb",
                                name=f"zrb_{l}_{g}")
                xph = psum.tile([H, 2, GRP, B], F32, tag="xph",
                                name=f"xph_{l}_{g}")

                for d in (0, 1):
                    for gi in range(3):
                        out_ap = (zrb[:, 2 * d + gi, :, :] if gi < 2
                                  else xph[:, d, :, :])
                        for k in range(2):
                            nc.tensor.matmul(
                                out_ap,
                                _r(w_ap(l, d, k, gi)),
                                _r(rhs[(d, k)]),
                                start=(k == 0), stop=False,
                                skip_group_check=True)
                        if has_bias:
                            nc.tensor.matmul(
                                out_ap,
                                _r(bias_ap(l, d, gi)),
                                _r(ones_sb),
                                start=False, stop=False,
                                skip_group_check=True)

                outbuf = outp.tile([H, 2, GRP, B], F32R, tag="outbuf",
                                   name=f"outbuf_{l}_{g}")

                for tl in range(GRP):
                    cb = GRP - 1 - tl
                    if prev_out is None and tl == 0:
                        hprev = h0_sb[:, :, :]
                        hp_f, hp_b = h0_sb[:, 0, :], h0_sb[:, 1, :]
                    elif tl == 0:
                        hprev = pair2(prev_out, GRP - 1, 0)
                        hp_f = prev_out[:, 0, GRP - 1, :]
                        hp_b = prev_out[:, 1, 0, :]
                    else:
                        hprev = pair2(outbuf, tl - 1, cb + 1)
                        hp_f = outbuf[:, 0, tl - 1, :]
                        hp_b = outbuf[:, 1, cb + 1, :]

                    scratch = pscr.tile([H, 2, B], F32, tag="scratch",
                                        name=f"scr_{l}_{g}_{tl}")
                    for d, hp_d, col in ((0, hp_f, tl), (1, hp_b, cb)):
                        for gi in range(3):
                            out_ap = (zrb[:, 2 * d + gi, col, :] if gi < 2
                                      else scratch[:, d, :])
                            nc.tensor.matmul(
                                out_ap,
                                _r(u_ap(l, d, gi)),
                                _r(hp_d),
                                start=(gi == 2 and d == 0), stop=True,
                                skip_group_check=True)
                    if has_bhh:
                        nc.tensor.matmul(
                            scratch[:, :, :], _r(bhh_ap(l)),
                            _r(ind2_sb), start=False, stop=True,
                            skip_group_check=True)

                    zrout = stepp.tile([H, 4, B], F32, tag="zrout",
                                       name=f"zrout_{l}_{g}_{tl}")
                    tt = stepp.tile([H, 2, B], F32, tag="tt",
                                    name=f"tt_{l}_{g}_{tl}")
                    arg = stepp.tile([H, 2, B], F32, tag="arg",
                                     name=f"arg_{l}_{g}_{tl}")
                    hh = stepp.tile([H, 2, B], F32, tag="hh",
                                    name=f"hh_{l}_{g}_{tl}")
                    dd = stepp.tile([H, 2, B], F32, tag="dd",
                                    name=f"dd_{l}_{g}_{tl}")
                    ee = stepp.tile([H, 2, B], F32, tag="ee",
                                    name=f"ee_{l}_{g}_{tl}")
                    dirs = ((0, hp_f, tl), (1, hp_b, cb))
                    stages = [
                        lambda d, hp_d, col: nc.scalar.activation(
                            zrout[:, 2 * d:2 * d + 2, :],
                            zrb[:, 2 * d:2 * d + 2, col, :], AF.Sigmoid),
                        lambda d, hp_d, col: nc.vector.tensor_mul(
                            tt[:, d, :], scratch[:, d, :],
                            zrout[:, 2 * d + 1, :]),
                        lambda d, hp_d, col: nc.vector.tensor_add(
                            arg[:, d, :], tt[:, d, :], xph[:, d, col, :]),
                        lambda d, hp_d, col: nc.scalar.activation(
                            hh[:, d, :], arg[:, d, :], AF.Tanh),
                        lambda d, hp_d, col: nc.vector.tensor_sub(
                            dd[:, d, :], hp_d, hh[:, d, :]),
                        lambda d, hp_d, col: nc.vector.tensor_mul(
                            ee[:, d, :], zrout[:, 2 * d, :], dd[:, d, :]),
                        lambda d, hp_d, col: nc.vector.tensor_add(
                            outbuf[:, d, col, :], ee[:, d, :], hh[:, d, :]),
                    ]
                    for stage in stages:
                        for d, hp_d, col in dirs:
                            stage(d, hp_d, col)

                if l < L - 1:
                    sf, sb = seqs[l % 2]
                    nc.sync.dma_start(
                        out=sf.rearrange("p (t b) -> p t b", b=B)[
                            :, GRP * g:GRP * (g + 1), :],
                        in_=outbuf[:, 0, :, :])
                    t_lo_b = T - GRP * (g + 1)
                    nc.sync.dma_start(
                        out=sb.rearrange("p (t b) -> p t b", b=B)[
                            :, t_lo_b:t_lo_b + GRP, :],
                        in_=outbuf[:, 1, :, :])
                prev_out = outbuf
            prev_out = None

        py = pscr.tile([1, B], F32, tag="scratch", name="py")
        nc.tensor.matmul(py, _r(wd_ap(0)),
                         _r(outbuf[:, 0, GRP - 1, :]),
                         start=True, stop=False, skip_group_check=True)
        nc.tensor.matmul(py, _r(wd_ap(1)),
                         _r(outbuf[:, 1, 0, :]),
                         start=False, stop=True, skip_group_check=True)
        y_sb = const.tile([1, B], F32)
        nc.scalar.activation(y_sb, py, AF.Sigmoid)
        nc.sync.dma_start(out=y[:], in_=y_sb)


def _prep_host(Ws, Us, bs, Wd, L, GRP, B_loc):
    Ws = np.asarray(Ws, np.float32)
    Us = np.asarray(Us, np.float32)
    bs = np.asarray(bs, np.float32)
    Wd = np.asarray(Wd, np.float32)
    has_bias = bool(np.any(bs != 0))
    has_bhh = bool(np.any(bs[:, :, 1, 2 * H:] != 0))
    CW = L * 2 * 2 * 3 * H
    CU = L * 2 * 3 * H
    GRPB = GRP * B_loc
    C = CW + CU + 2 + CU + L * H + GRPB + 4 * B_loc
    pack = np.zeros((H, C), np.float32)
    c_ones = CW + 2 * CU + 2 + L * H
    pack[0, c_ones:c_ones + GRPB] = 1.0
    pack[0, c_ones + GRPB:c_ones + GRPB + B_loc] = 1.0
    pack[1, c_ones + GRPB + B_loc:c_ones + GRPB + 2 * B_loc] = 1.0
    pack[:, :CW] = (Ws.reshape(L, 2, 2, H, 3 * H)
                    .transpose(3, 0, 1, 2, 4).reshape(H, CW))
    pack[:, CW:CW + CU] = (Us.transpose(2, 0, 1, 3).reshape(H, CU))
    pack[:, CW + CU] = Wd[0:H, 0]
    pack[:, CW + CU + 1] = Wd[H:2 * H, 0]
    if has_bias:
        bsum = bs[:, :, 0, :].copy()
        bsum[:, :, :2 * H] += bs[:, :, 1, :2 * H]
        pack[0, CW + CU + 2:CW + 2 * CU + 2] = bsum.reshape(-1)
    if has_bhh:
        cb = CW + 2 * CU + 2
        pack[0:2, cb:cb + L * H] = np.transpose(
            bs[:, :, 1, 2 * H:], (1, 0, 2)).reshape(2, L * H)
    return {"wpack": pack}, has_bias, has_bhh


def run_gru(x, Ws, Us, bs, Wd, bd, n_cores=N_CORES, L=3, GRP=16, trace=False):
    x = np.ascontiguousarray(np.asarray(x, np.float32))
    B_full, T, _ = x.shape
    B_loc = B_full // n_cores
    common, has_bias, has_bhh = _prep_host(Ws, Us, bs, Wd, L, GRP, B_loc)

    nc = bacc.Bacc()
    build_gru(nc, B_loc, T, L, GRP, has_bias, has_bhh)
    nc.compile()

    in_maps = []
    for c in range(n_cores):
        m = dict(common)
        xs = x[c * B_loc:(c + 1) * B_loc]
        m["x"] = np.ascontiguousarray(
            xs.transpose(2, 1, 0).reshape(D_IN, T * B_loc))
        in_maps.append(m)

    res = run_bass_kernel_spmd(nc, in_maps, core_ids=list(range(n_cores)),
                               trace=trace)
    parts = [res.results[c]["y"][0] for c in range(n_cores)]
    out = np.concatenate(parts).reshape(B_full, 1).astype(np.float32)
    return out, res


def kernel(x, Ws, Us, bs, Wd, bd):
    bd = np.asarray(bd, np.float32).reshape(-1)
    out, _ = run_gru(x, Ws, Us, bs, Wd, bd)
    if np.any(bd != 0):
        p = np.clip(np.float64(out), 1e-12, 1 - 1e-12)
        out = (1.0 / (1.0 + np.exp(-(np.log(p / (1 - p)) + bd[0]))))
    return np.asarray(out, np.float32)

